# revision 13
# baseline (speedup 1.0000x reference)
import numpy as np
import ml_dtypes

import concourse.bacc as bacc
import concourse.tile as tile
from concourse import mybir

# Problem: NIMSCrossEntropyLoss
#   preds (4, 4, 4, 512, 512) f32, targets (4, 4, 512, 512) int32
#   Only the S=-1 slice contributes:
#   loss = [sum_pixels logsumexp_c(p) - sum_pixels p[target]] / N_BATCH
# Shard the 4*512*512 = 1048576 pixels over 8 cores:
#   131072 pixels/core as [128 partitions, 1024 free] channel planes (bf16).
# v3: per-plane DRAM tensors + 3 parallel DMA queues (ACT/SP/SWDGE) +
#     per-plane exp and a DVE order that feeds ln as early as possible.

N_CORES = 8
P = 128           # partitions
C = 4             # classes
N_BATCH = 4       # reference divides by this
F = 1024          # pixels per partition per core

BF16 = mybir.dt.bfloat16
F32 = mybir.dt.float32

_PATCHED = False


def _patch_act_tables():
    """Force exp+ln into the combined ACT table so only one table load is
    emitted (greedy per-function set choice otherwise alternates sets)."""
    global _PATCHED
    if _PATCHED:
        return
    import concourse.hw_specs as hw_specs
    real = hw_specs.get_activation_tables
    Exp = mybir.ActivationFunctionType.Exp
    Ln = mybir.ActivationFunctionType.Ln

    def patched(arch):
        out = {}
        for name, fns in dict(real(arch)).items():
            if name != "natural_log_exp_and_others":
                fns = fns - {Exp, Ln}
            out[name] = fns
        return out

    bacc.get_activation_tables = patched
    _PATCHED = True


def build_nc(f=F, finalize=True):
    """One core's shard: p0..p3 channel planes [P, f] bf16, tgt [P, f] bf16;
    out [P, 5] f32 = per-partition sums (p_t for c=0..3, lse)."""
    _patch_act_tables()
    nc = bacc.Bacc("TRN2", target_bir_lowering=False, debug=False)
    planes = [nc.dram_tensor(f"p{c}", (P, f), BF16, kind="ExternalInput").ap()
              for c in range(C)]
    tgt = nc.dram_tensor("tgt", (P, f), BF16, kind="ExternalInput").ap()
    out = nc.dram_tensor("out", (P, 5), F32, kind="ExternalOutput").ap()

    Exp = mybir.ActivationFunctionType.Exp
    Ln = mybir.ActivationFunctionType.Ln

    with tile.TileContext(nc) as tc:
        with tc.tile_pool(name="w", bufs=1) as w:
            pt = [w.tile([P, f], BF16, name=f"pt{c}") for c in range(C)]
            tt = w.tile([P, f], BF16)

            # Sync + GpSimd DMA queues only: scalar.dma_start forces a
            # spurious extra ACT table load whose DRAM traffic starves the
            # input DMAs. tgt and p0 go first as half-transfers split across
            # both queues so stt0/exp0 can start ~1us earlier.
            h = f // 2
            nc.sync.dma_start(out=tt[:, 0:h], in_=tgt[:, 0:h])
            nc.gpsimd.dma_start(out=tt[:, h:f], in_=tgt[:, h:f])
            nc.sync.dma_start(out=pt[0][:, 0:h], in_=planes[0][:, 0:h])
            nc.gpsimd.dma_start(out=pt[0][:, h:f], in_=planes[0][:, h:f])
            nc.sync.dma_start(out=pt[1], in_=planes[1])
            nc.gpsimd.dma_start(out=pt[2], in_=planes[2])
            nc.sync.dma_start(out=pt[3], in_=planes[3])

            res = w.tile([P, 5], F32)
            e = [w.tile([P, f], BF16, name=f"e{c}") for c in range(C)]
            for c in range(C):
                nc.scalar.activation(out=e[c], in_=pt[c], func=Exp)

            scr = w.tile([P, 4 * f], BF16)

            def stt(c):
                nc.vector.scalar_tensor_tensor(
                    out=scr[:, c * f:(c + 1) * f], in0=tt, scalar=float(c),
                    in1=pt[c],
                    op0=mybir.AluOpType.is_equal, op1=mybir.AluOpType.mult,
                    accum_out=res[:, c:c + 1],
                )

            s01 = w.tile([P, f], BF16)
            s012 = w.tile([P, f], BF16)
            s = w.tile([P, f], BF16)

            # The scheduler batches all 4 stts first on DVE regardless of
            # emission order, so shorten the post-stt tail: s01 runs on
            # gpsimd (slow but fully overlapped with the stts), leaving only
            # s23 and s on DVE before ln.
            stt(0)
            stt(1)
            nc.gpsimd.tensor_tensor(out=s01, in0=e[0], in1=e[1],
                                    op=mybir.AluOpType.add)
            stt(2)
            stt(3)
            nc.vector.tensor_tensor(out=s012, in0=e[2], in1=e[3],
                                    op=mybir.AluOpType.add)
            nc.vector.tensor_tensor(out=s, in0=s01, in1=s012,
                                    op=mybir.AluOpType.add)

            lnout = w.tile([P, f], BF16)
            with tc.high_priority():
                nc.scalar.activation(out=lnout, in_=s, func=Ln,
                                     accum_out=res[:, 4:5])

            nc.sync.dma_start(out=out, in_=res)
    if finalize:
        nc.finalize()
    return nc


_NC_CACHE = {}


def _get_nc(f=F):
    if f not in _NC_CACHE:
        _NC_CACHE[f] = build_nc(f)
    return _NC_CACHE[f]


def prep_inputs(preds, targets):
    """Host-side shard prep: S=-1 slice, per-channel planes, 8-way split."""
    p = np.asarray(preds)[:, -1]       # (N=4, C=4, 512, 512) f32
    t = np.asarray(targets)[:, -1]     # (4, 512, 512) int
    arr = np.transpose(p, (1, 0, 2, 3)).reshape(C, N_CORES, P, -1)
    arr = arr.astype(ml_dtypes.bfloat16)
    tf = t.reshape(N_CORES, P, -1).astype(ml_dtypes.bfloat16)
    maps = []
    for k in range(N_CORES):
        m = {f"p{c}": np.ascontiguousarray(arr[c, k]) for c in range(C)}
        m["tgt"] = tf[k]
        maps.append(m)
    return maps


def reduce_outputs(results):
    total = 0.0
    for d in results:
        o = d["out"].astype(np.float64)
        total += float(o[:, 4].sum() - o[:, 0:4].sum())
    return np.float32(total / N_BATCH)


def kernel(preds, targets, _trace=False, _trace_kwargs=None):
    from concourse.bass_utils import run_bass_kernel_spmd

    in_maps = prep_inputs(preds, targets)
    f = in_maps[0]["tgt"].shape[1]
    nc = _get_nc(f=f)
    r = run_bass_kernel_spmd(
        nc, in_maps, core_ids=list(range(N_CORES)),
        trace=_trace, **(_trace_kwargs or {}),
    )
    kernel.last_run = r
    return reduce_outputs(r.results)


kernel.last_run = None


# revision 15
# speedup vs baseline: 1.0830x; 1.0830x over previous
import numpy as np
import ml_dtypes

import concourse.bacc as bacc
import concourse.tile as tile
from concourse import mybir

# Problem: NIMSCrossEntropyLoss
#   preds (4, 4, 4, 512, 512) f32, targets (4, 4, 512, 512) int32
#   Only the S=-1 slice contributes:
#   loss = [sum_pixels logsumexp_c(p) - sum_pixels p[target]] / N_BATCH
# Shard the 4*512*512 = 1048576 pixels over 8 cores:
#   131072 pixels/core as [128 partitions, 1024 free] channel planes (bf16).
# v3: per-plane DRAM tensors + 3 parallel DMA queues (ACT/SP/SWDGE) +
#     per-plane exp and a DVE order that feeds ln as early as possible.

N_CORES = 8
P = 128           # partitions
C = 4             # classes
N_BATCH = 4       # reference divides by this
F = 1024          # pixels per partition per core

BF16 = mybir.dt.bfloat16
F32 = mybir.dt.float32

_PATCHED = False


def _patch_act_tables():
    """Force exp+ln into the combined ACT table so only one table load is
    emitted (greedy per-function set choice otherwise alternates sets)."""
    global _PATCHED
    if _PATCHED:
        return
    import concourse.hw_specs as hw_specs
    real = hw_specs.get_activation_tables
    Exp = mybir.ActivationFunctionType.Exp
    Ln = mybir.ActivationFunctionType.Ln

    def patched(arch):
        out = {}
        for name, fns in dict(real(arch)).items():
            if name != "natural_log_exp_and_others":
                fns = fns - {Exp, Ln}
            out[name] = fns
        return out

    bacc.get_activation_tables = patched
    _PATCHED = True


def build_nc(f=F, finalize=True):
    """One core's shard: p0..p3 channel planes [P, f] bf16, tgt [P, f] bf16;
    out [P, 5] f32 = per-partition sums (p_t for c=0..3, lse)."""
    _patch_act_tables()
    nc = bacc.Bacc("TRN2", target_bir_lowering=False, debug=False)
    planes = [nc.dram_tensor(f"p{c}", (P, f), BF16, kind="ExternalInput").ap()
              for c in range(C)]
    tgt = nc.dram_tensor("tgt", (P, f), BF16, kind="ExternalInput").ap()
    out = nc.dram_tensor("out", (P, 5), F32, kind="ExternalOutput").ap()

    Exp = mybir.ActivationFunctionType.Exp
    Ln = mybir.ActivationFunctionType.Ln

    with tile.TileContext(nc) as tc:
        with tc.tile_pool(name="w", bufs=1) as w:
            pt = [w.tile([P, f], BF16, name=f"pt{c}") for c in range(C)]
            tt = w.tile([P, f], BF16)

            # Sync + GpSimd DMA queues only: scalar.dma_start forces a
            # spurious extra ACT table load whose DRAM traffic starves the
            # input DMAs. Interleaved completion -> p0, tgt, p1, p2, p3.
            # (Splitting tgt/p0 into half-transfers was tried and is slower:
            # extra issue overhead pushes the ACT table load late, and a
            # concurrent gpsimd add causes SBUF contention that slows DVE.)
            nc.sync.dma_start(out=pt[0], in_=planes[0])
            nc.gpsimd.dma_start(out=tt, in_=tgt)
            nc.sync.dma_start(out=pt[1], in_=planes[1])
            nc.gpsimd.dma_start(out=pt[2], in_=planes[2])
            nc.sync.dma_start(out=pt[3], in_=planes[3])

            res = w.tile([P, 5], F32)
            e = [w.tile([P, f], BF16, name=f"e{c}") for c in range(C)]
            for c in range(C):
                nc.scalar.activation(out=e[c], in_=pt[c], func=Exp)

            scr = w.tile([P, 4 * f], BF16)

            def stt(c):
                nc.vector.scalar_tensor_tensor(
                    out=scr[:, c * f:(c + 1) * f], in0=tt, scalar=float(c),
                    in1=pt[c],
                    op0=mybir.AluOpType.is_equal, op1=mybir.AluOpType.mult,
                    accum_out=res[:, c:c + 1],
                )

            s01 = w.tile([P, f], BF16)
            s012 = w.tile([P, f], BF16)
            s = w.tile([P, f], BF16)

            # The scheduler batches all 4 stts first on DVE regardless of
            # emission order (priority hints don't change it), then runs the
            # three adds and ln.
            stt(0)
            stt(1)
            nc.vector.tensor_tensor(out=s01, in0=e[0], in1=e[1],
                                    op=mybir.AluOpType.add)
            stt(2)
            nc.vector.tensor_tensor(out=s012, in0=s01, in1=e[2],
                                    op=mybir.AluOpType.add)
            nc.vector.tensor_tensor(out=s, in0=s012, in1=e[3],
                                    op=mybir.AluOpType.add)
            stt(3)

            lnout = w.tile([P, f], BF16)
            nc.scalar.activation(out=lnout, in_=s, func=Ln,
                                 accum_out=res[:, 4:5])

            nc.sync.dma_start(out=out, in_=res)
    if finalize:
        nc.finalize()
    return nc


_NC_CACHE = {}


def _get_nc(f=F):
    if f not in _NC_CACHE:
        _NC_CACHE[f] = build_nc(f)
    return _NC_CACHE[f]


def prep_inputs(preds, targets):
    """Host-side shard prep: S=-1 slice, per-channel planes, 8-way split."""
    p = np.asarray(preds)[:, -1]       # (N=4, C=4, 512, 512) f32
    t = np.asarray(targets)[:, -1]     # (4, 512, 512) int
    arr = np.transpose(p, (1, 0, 2, 3)).reshape(C, N_CORES, P, -1)
    arr = arr.astype(ml_dtypes.bfloat16)
    tf = t.reshape(N_CORES, P, -1).astype(ml_dtypes.bfloat16)
    maps = []
    for k in range(N_CORES):
        m = {f"p{c}": np.ascontiguousarray(arr[c, k]) for c in range(C)}
        m["tgt"] = tf[k]
        maps.append(m)
    return maps


def reduce_outputs(results):
    total = 0.0
    for d in results:
        o = d["out"].astype(np.float64)
        total += float(o[:, 4].sum() - o[:, 0:4].sum())
    return np.float32(total / N_BATCH)


def kernel(preds, targets, _trace=False, _trace_kwargs=None):
    from concourse.bass_utils import run_bass_kernel_spmd

    in_maps = prep_inputs(preds, targets)
    f = in_maps[0]["tgt"].shape[1]
    nc = _get_nc(f=f)
    r = run_bass_kernel_spmd(
        nc, in_maps, core_ids=list(range(N_CORES)),
        trace=_trace, **(_trace_kwargs or {}),
    )
    kernel.last_run = r
    return reduce_outputs(r.results)


kernel.last_run = None
